# revision 14
# baseline (speedup 1.0000x reference)
"""Contrastive-learning NCE loss on 8 trn2 NeuronCores (Bass/Tile).

Problem (hardcoded shapes): B=8, L=1024, D_in=512, D_feat=256, N=B*L=8192.
  emb_k = relu(feature_k @ W + b)                     [B, L, Df]
  positive = <e1,e2> + banded_diag_mean terms         [N]
  negative = logsumexp(e1 @ e2.T, axis=-1) - log(N)   [N]
  loss = mean(-positive + negative)

Sharding: token dim N split across 8 cores = one batch row each (L == N/8).
Each core computes its [1024, 8192] slab of the similarity matrix against the
full emb_2 (recomputed locally from full feature2). The host rotates feature2
per core so the core's own batch always sits at columns 0:1023 -> the SPMD
program is core-index free.

Device layout is d-major ("transposed"): embT[d, token] so the PE contracts
over d for both the projection (K=D_in) and the sim matmul (K=Df).
Matmul operands are bf16 (full PE rate); PSUM accumulation is fp32.

logsumexp per row with shift C = diag = <e1_m, e2_m> (exact for any C; C is
a member of the row so sum >= 1, and overflow would need an off-diagonal
dot 88 above the diagonal one -- impossible at these scales). ACT computes
exp(psum - C) with fused per-row accumulation; host finishes lse = C + log(S).
"""

import numpy as np
import ml_dtypes
from contextlib import ExitStack

import concourse.bass as bass
import concourse.tile as tile
from concourse import bacc, mybir
from concourse import bass_utils

dt = mybir.dt
AF = mybir.ActivationFunctionType
ALU = mybir.AluOpType

N_CORES = 8
B, L, DIN, DF = 8, 1024, 512, 256
N = B * L
KO = DIN // 128     # 4 k-tiles of the projection contraction
NDT = DF // 128     # 2 d-tiles of the embedding dim
PAD = 4             # box-filter padding (max supported positive_range)
CW = 2048           # sim-phase column group (one PSUM tile / one ACT)

_module_cache = {}

# NOTE: walrus's LDWEIGHTS-elision pass (--enable-ldw-opt) was tried to elide
# the redundant per-matmul weight loads (~100ns each on the PE front-end), but
# that pass crashes codegen (visitInstLdweights) in this toolchain build, so
# the per-matmul LDWEIGHTS cost stays.


def _box_terms(w: int):
    """Decompose window width w (odd, <= 2*PAD+1) into power-of-2 segments:
    returns [(pow, offset), ...] s.t. window = concat of segments."""
    terms, off = [], 0
    for p in (8, 4, 2, 1):
        if w >= p:
            terms.append((p, off))
            off += p
            w -= p
    assert w == 0
    return terms


def _build(r_self: int, r_tgt: int):
    nc = bacc.Bacc("TRN2", target_bir_lowering=False, debug=False, num_devices=N_CORES)

    f1t = nc.dram_tensor("f1t", [DIN, L], dt.bfloat16, kind="ExternalInput").ap()
    f2t = nc.dram_tensor("f2t", [DIN, N], dt.bfloat16, kind="ExternalInput").ap()
    w_in = nc.dram_tensor("w_in", [DIN, DF], dt.bfloat16, kind="ExternalInput").ap()
    b_in = nc.dram_tensor("b_in", [DF], dt.float32, kind="ExternalInput").ap()

    pos_main = nc.dram_tensor("pos_main", [L], dt.float32, kind="ExternalOutput").ap()
    pos_self = nc.dram_tensor("pos_self", [L], dt.float32, kind="ExternalOutput").ap()
    pos_tgt = nc.dram_tensor("pos_tgt", [L], dt.float32, kind="ExternalOutput").ap()
    s_out = nc.dram_tensor("s_out", [128, 8 * (N // CW)], dt.float32, kind="ExternalOutput").ap()

    with tile.TileContext(nc) as tc, ExitStack() as ctx:
        const = ctx.enter_context(tc.tile_pool(name="const", bufs=1))
        stage = ctx.enter_context(tc.tile_pool(name="stage", bufs=2))
        emb = ctx.enter_context(tc.tile_pool(name="emb", bufs=1))
        band = ctx.enter_context(tc.tile_pool(name="band", bufs=1))
        prodp = ctx.enter_context(tc.tile_pool(name="prodp", bufs=2))
        rows = ctx.enter_context(tc.tile_pool(name="rows", bufs=1))
        esc = ctx.enter_context(tc.tile_pool(name="esc2", bufs=2))
        small = ctx.enter_context(tc.tile_pool(name="small", bufs=2))
        mmp = ctx.enter_context(tc.tile_pool(name="mmp", bufs=2, space="PSUM"))

        # ---- constants -------------------------------------------------
        wt = const.tile([128, KO * DF], dt.bfloat16)       # W as [k%128, (ko d)]
        nc.sync.dma_start(out=wt[:].rearrange("p (ko d) -> p ko d", ko=KO),
                          in_=w_in[:].rearrange("(ko p) d -> p ko d", p=128))
        b_col = const.tile([128, NDT], dt.float32)         # bias per (d%128, dtile)
        nc.sync.dma_start(out=b_col[:], in_=b_in[:].rearrange("(d p) -> p d", p=128))
        ones_f = const.tile([128, 1], dt.float32)
        nc.vector.memset(ones_f[:], 1.0)
        ones = const.tile([128, 1], dt.bfloat16)
        nc.vector.tensor_copy(ones[:], ones_f[:])

        # ---- projection: embT[d, tok] = relu(W.T @ fT + b) -------------
        e1 = [emb.tile([128, L], dt.bfloat16, name=f"e1_{d}", tag=f"e1_{d}")
              for d in range(NDT)]
        e2 = [emb.tile([128, N], dt.bfloat16, name=f"e2_{d}", tag=f"e2_{d}")
              for d in range(NDT)]

        def project(src_ap, col0, ncols, dst, dst_col0):
            """relu-project fT columns [col0, col0+ncols) into dst[dtile][:, dst_col0...]."""
            fst = stage.tile([128, KO * ncols], dt.bfloat16, tag="fstage")
            nc.sync.dma_start(
                out=fst[:].rearrange("p (ko n) -> p ko n", ko=KO),
                in_=src_ap[:, col0:col0 + ncols].rearrange("(ko p) n -> p ko n", p=128))
            for d in range(NDT):
                ps = mmp.tile([128, ncols], dt.float32, tag="mm")
                for ko in range(KO):                 # weight-stationary inner order
                    for half in range(ncols // 512):
                        nc.tensor.matmul(
                            ps[:, half * 512:(half + 1) * 512],
                            wt[:, ko * DF + d * 128: ko * DF + d * 128 + 128],
                            fst[:, ko * ncols + half * 512: ko * ncols + (half + 1) * 512],
                            start=(ko == 0), stop=(ko == KO - 1))
                nc.vector.tensor_scalar(
                    dst[d][:, dst_col0:dst_col0 + ncols], ps[:],
                    b_col[:, d:d + 1], 0.0, ALU.add, ALU.max)

        # split the first projections into 512-col pieces so the PE starts
        # as soon as the first quarter-stage DMA lands (trims the idle head)
        project(f1t, 0, 512, e1, 0)
        project(f1t, 512, 512, e1, 512)
        project(f2t, 0, 512, e2, 0)        # own batch first (banded phase needs it)
        project(f2t, 512, 512, e2, 512)

        # ---- banded positive terms (own batch = e2 cols 0:L) -----------
        def boxsum(src_view, r, tag):
            """Return [128, L] view/tile: out[:, j] = sum_{|d|<=r} src[:, j+d] (clipped)."""
            wdt = 2 * r + 1
            pb = band.tile([128, L + 2 * PAD], dt.bfloat16, name=f"pb_{tag}",
                           tag="pb", bufs=2)
            nc.vector.memzero(pb[:])
            nc.vector.tensor_copy(pb[:, PAD:PAD + L], src_view)
            s = {1: pb}
            for p in (2, 4, 8):
                if wdt >= p:
                    sp = band.tile([128, L + 2 * PAD], dt.bfloat16, name=f"s{p}_{tag}",
                                   tag=f"s{p}")
                    h = p // 2
                    n_valid = L + 2 * PAD - p + 1
                    nc.vector.tensor_tensor(
                        sp[:, :n_valid], s[h][:, :n_valid], s[h][:, h:h + n_valid], ALU.add)
                    s[p] = sp
            terms = _box_terms(wdt)
            t0 = PAD - r
            if len(terms) == 1:
                p0, o0 = terms[0]
                return s[p0][:, t0 + o0: t0 + o0 + L]
            acc = band.tile([128, L], dt.bfloat16, name=f"box_{tag}", tag="box", bufs=6)
            p0, o0 = terms[0]
            p1, o1 = terms[1]
            nc.vector.tensor_tensor(acc[:], s[p0][:, t0 + o0: t0 + o0 + L],
                                    s[p1][:, t0 + o1: t0 + o1 + L], ALU.add)
            for p, o in terms[2:]:
                nc.vector.tensor_tensor(acc[:], acc[:], s[p][:, t0 + o: t0 + o + L], ALU.add)
            return acc[:]

        def reduce_group(pairs, out_dram, tag):
            """out_dram[j] = sum over pairs (a,b) and d of (a*b)[d, j]."""
            row = rows.tile([1, L], dt.float32, tag=f"row_{tag}")
            for half in range(L // 512):
                rp = mmp.tile([1, 512], dt.float32, tag="mm", name=f"rp_{tag}_{half}")
                for gi, (a_view, b_view) in enumerate(pairs):
                    prod = prodp.tile([128, 512], dt.bfloat16, tag="prod")
                    nc.vector.tensor_tensor(
                        prod[:], a_view[:, half * 512:(half + 1) * 512],
                        b_view[:, half * 512:(half + 1) * 512], ALU.mult)
                    nc.tensor.matmul(rp[:], ones[:], prod[:],
                                     start=(gi == 0), stop=(gi == len(pairs) - 1))
                nc.vector.tensor_copy(row[:, half * 512:(half + 1) * 512], rp[:])
            nc.sync.dma_start(out=out_dram[:].rearrange("(one n) -> one n", one=1), in_=row[:])

        e2L = [e2[d][:, 0:L] for d in range(NDT)]
        reduce_group([(e1[d][:], e2L[d]) for d in range(NDT)], pos_main, "main")
        if r_self > 0:
            bx1 = [boxsum(e1[d][:], r_self, f"s1_{d}") for d in range(NDT)]
            bx2 = [boxsum(e2L[d], r_self, f"s2_{d}") for d in range(NDT)]
            reduce_group([(e1[d][:], bx1[d]) for d in range(NDT)]
                         + [(e2L[d], bx2[d]) for d in range(NDT)], pos_self, "self")
        else:
            zr = rows.tile([1, L], dt.float32, tag="zr")
            nc.vector.memset(zr[:], 0.0)
            nc.sync.dma_start(out=pos_self[:].rearrange("(one n) -> one n", one=1), in_=zr[:])
        if r_tgt > 0:
            bxt = [boxsum(e2L[d], r_tgt, f"t_{d}") for d in range(NDT)]
            reduce_group([(e1[d][:], bxt[d]) for d in range(NDT)], pos_tgt, "tgt")
        else:
            zr2 = rows.tile([1, L], dt.float32, tag="zr2")
            nc.vector.memset(zr2[:], 0.0)
            nc.sync.dma_start(out=pos_tgt[:].rearrange("(one n) -> one n", one=1), in_=zr2[:])

        # diag bias column layout: [128, 8] with diag[p, a] = pos_main[a*128+p]
        neg_diag = const.tile([128, 8], dt.float32)
        nc.sync.dma_start(out=neg_diag[:], in_=pos_main[:].rearrange("(a p) -> p a", p=128))
        nc.vector.tensor_scalar_mul(neg_diag[:], neg_diag[:], -1.0)

        # ---- rest of e2 projection (chunks 1..7) ------------------------
        for c in range(1, B):
            project(f2t, c * L, L, e2, c * L)

        # ---- sim slab + streaming exp-sum -------------------------------
        ncg = N // CW                           # column groups per row tile
        stot = const.tile([128, 8 * ncg], dt.float32)
        for m in range(8):                      # 128-token row tiles
            for c in range(ncg):                # CW-wide column groups
                ps = mmp.tile([128, CW], dt.float32, tag="mm")
                for d_ in range(NDT):           # weight-stationary inner order
                    for q in range(CW // 512):
                        nc.tensor.matmul(
                            ps[:, q * 512:(q + 1) * 512],
                            e1[d_][:, m * 128:(m + 1) * 128],
                            e2[d_][:, c * CW + q * 512: c * CW + (q + 1) * 512],
                            start=(d_ == 0), stop=(d_ == NDT - 1))
                ex = esc.tile([128, CW], dt.bfloat16, tag="ex")
                nc.scalar.activation(ex[:], ps[:], AF.Exp,
                                     bias=neg_diag[:, m:m + 1], scale=1.0,
                                     accum_out=stot[:, m * ncg + c: m * ncg + c + 1])
        nc.sync.dma_start(out=s_out[:], in_=stot[:])

    nc.compile()
    return nc


def kernel(feature1, feature2, W, b, positive_range_self, positive_range_tgt):
    r_self = int(np.asarray(positive_range_self))
    r_tgt = int(np.asarray(positive_range_tgt))
    assert 0 <= r_self <= PAD and 0 <= r_tgt <= PAD

    key = (r_self, r_tgt)
    if key not in _module_cache:
        _module_cache[key] = _build(r_self, r_tgt)
    nc = _module_cache[key]

    in_maps = _make_in_maps(feature1, feature2, W, b)
    res = bass_utils.run_bass_kernel_spmd(nc, in_maps, list(range(N_CORES)))

    # ---- host combine (fp64) ---------------------------------------------
    j = np.arange(L)
    loss_terms = []
    for i in range(N_CORES):
        r = res.results[i]
        # S groups: stot[p, m*ncg + c]; token j = m*128 + p; sum over c groups
        ncg = N // CW
        S = r["s_out"].astype(np.float64).reshape(128, 8, ncg).sum(axis=2)
        S = S.T.reshape(L)                                   # token j at [j%128, j//128]
        t = np.log(S) - np.log(float(N))                     # negative - diag (diag cancels)
        if r_self > 0:
            cnt = np.minimum(L - 1, j + r_self) - np.maximum(0, j - r_self) + 1.0
            t -= r["pos_self"].astype(np.float64) / cnt
        if r_tgt > 0:
            cnt = np.minimum(L - 1, j + r_tgt) - np.maximum(0, j - r_tgt) + 1.0
            t -= r["pos_tgt"].astype(np.float64) / cnt
        loss_terms.append(t)
    loss = np.mean(np.concatenate(loss_terms))
    return np.float32(loss)


def _make_in_maps(feature1, feature2, W, b):
    bf16 = ml_dtypes.bfloat16
    f1 = np.asarray(feature1, dtype=np.float32)
    f2 = np.asarray(feature2, dtype=np.float32)
    Wr = np.ascontiguousarray(np.asarray(W, dtype=np.float32).astype(bf16))
    bv = np.ascontiguousarray(np.asarray(b, dtype=np.float32))
    f2t_full = f2.reshape(N, DIN).T.astype(bf16)             # [DIN, N]
    in_maps = []
    for i in range(N_CORES):
        f1t_i = np.ascontiguousarray(f1[i].T.astype(bf16))   # [DIN, L]
        f2t_rot = np.ascontiguousarray(np.roll(f2t_full, -i * L, axis=1))
        in_maps.append({"f1t": f1t_i, "f2t": f2t_rot, "w_in": Wr, "b_in": bv})
    return in_maps


# revision 15
# speedup vs baseline: 1.0223x; 1.0223x over previous
"""Contrastive-learning NCE loss on 8 trn2 NeuronCores (Bass/Tile).

Problem (hardcoded shapes): B=8, L=1024, D_in=512, D_feat=256, N=B*L=8192.
  emb_k = relu(feature_k @ W + b)                     [B, L, Df]
  positive = <e1,e2> + banded_diag_mean terms         [N]
  negative = logsumexp(e1 @ e2.T, axis=-1) - log(N)   [N]
  loss = mean(-positive + negative)

Sharding: token dim N split across 8 cores = one batch row each (L == N/8).
Each core computes its [1024, 8192] slab of the similarity matrix against the
full emb_2 (recomputed locally from full feature2). The host rotates feature2
per core so the core's own batch always sits at columns 0:1023 -> the SPMD
program is core-index free.

Device layout is d-major ("transposed"): embT[d, token] so the PE contracts
over d for both the projection (K=D_in) and the sim matmul (K=Df).
Matmul operands are bf16 (full PE rate); PSUM accumulation is fp32.

logsumexp per row with shift C = diag = <e1_m, e2_m> (exact for any C; C is
a member of the row so sum >= 1, and overflow would need an off-diagonal
dot 88 above the diagonal one -- impossible at these scales). ACT computes
exp(psum - C) with fused per-row accumulation; host finishes lse = C + log(S).
"""

import numpy as np
import ml_dtypes
from contextlib import ExitStack

import concourse.bass as bass
import concourse.tile as tile
from concourse import bacc, mybir
from concourse import bass_utils

dt = mybir.dt
AF = mybir.ActivationFunctionType
ALU = mybir.AluOpType

N_CORES = 8
B, L, DIN, DF = 8, 1024, 512, 256
N = B * L
KO = DIN // 128     # 4 k-tiles of the projection contraction
NDT = DF // 128     # 2 d-tiles of the embedding dim
PAD = 4             # box-filter padding (max supported positive_range)
CW = 2048           # sim-phase column group (one PSUM tile / one ACT)

_module_cache = {}

# NOTE: walrus's LDWEIGHTS-elision pass (--enable-ldw-opt) was tried to elide
# the redundant per-matmul weight loads (~100ns each on the PE front-end), but
# that pass crashes codegen (visitInstLdweights) in this toolchain build, so
# the per-matmul LDWEIGHTS cost stays.


def _box_terms(w: int):
    """Decompose window width w (odd, <= 2*PAD+1) into power-of-2 segments:
    returns [(pow, offset), ...] s.t. window = concat of segments."""
    terms, off = [], 0
    for p in (8, 4, 2, 1):
        if w >= p:
            terms.append((p, off))
            off += p
            w -= p
    assert w == 0
    return terms


def _build(r_self: int, r_tgt: int):
    nc = bacc.Bacc("TRN2", target_bir_lowering=False, debug=False, num_devices=N_CORES)

    f1t = nc.dram_tensor("f1t", [DIN, L], dt.bfloat16, kind="ExternalInput").ap()
    f2t = nc.dram_tensor("f2t", [DIN, N], dt.bfloat16, kind="ExternalInput").ap()
    w_in = nc.dram_tensor("w_in", [DIN, DF], dt.bfloat16, kind="ExternalInput").ap()
    b_in = nc.dram_tensor("b_in", [DF], dt.float32, kind="ExternalInput").ap()

    pos_main = nc.dram_tensor("pos_main", [L], dt.float32, kind="ExternalOutput").ap()
    pos_self = nc.dram_tensor("pos_self", [L], dt.float32, kind="ExternalOutput").ap()
    pos_tgt = nc.dram_tensor("pos_tgt", [L], dt.float32, kind="ExternalOutput").ap()
    s_out = nc.dram_tensor("s_out", [128, 8 * (N // CW)], dt.float32, kind="ExternalOutput").ap()

    with tile.TileContext(nc) as tc, ExitStack() as ctx:
        const = ctx.enter_context(tc.tile_pool(name="const", bufs=1))
        stage = ctx.enter_context(tc.tile_pool(name="stage", bufs=2))
        emb = ctx.enter_context(tc.tile_pool(name="emb", bufs=1))
        band = ctx.enter_context(tc.tile_pool(name="band", bufs=1))
        prodp = ctx.enter_context(tc.tile_pool(name="prodp", bufs=2))
        rows = ctx.enter_context(tc.tile_pool(name="rows", bufs=1))
        esc = ctx.enter_context(tc.tile_pool(name="esc2", bufs=2))
        small = ctx.enter_context(tc.tile_pool(name="small", bufs=2))
        mmp = ctx.enter_context(tc.tile_pool(name="mmp", bufs=2, space="PSUM"))

        # ---- constants -------------------------------------------------
        wt = const.tile([128, KO * DF], dt.bfloat16)       # W as [k%128, (ko d)]
        nc.sync.dma_start(out=wt[:].rearrange("p (ko d) -> p ko d", ko=KO),
                          in_=w_in[:].rearrange("(ko p) d -> p ko d", p=128))
        b_col = const.tile([128, NDT], dt.float32)         # bias per (d%128, dtile)
        nc.sync.dma_start(out=b_col[:], in_=b_in[:].rearrange("(d p) -> p d", p=128))
        ones_f = const.tile([128, 1], dt.float32)
        nc.vector.memset(ones_f[:], 1.0)
        ones = const.tile([128, 1], dt.bfloat16)
        nc.vector.tensor_copy(ones[:], ones_f[:])

        # ---- projection: embT[d, tok] = relu(W.T @ fT + b) -------------
        e1 = [emb.tile([128, L], dt.bfloat16, name=f"e1_{d}", tag=f"e1_{d}")
              for d in range(NDT)]
        e2 = [emb.tile([128, N], dt.bfloat16, name=f"e2_{d}", tag=f"e2_{d}")
              for d in range(NDT)]

        def project(src_ap, col0, ncols, dst, dst_col0):
            """relu-project fT columns [col0, col0+ncols) into dst[dtile][:, dst_col0...]."""
            fst = stage.tile([128, KO * ncols], dt.bfloat16, tag="fstage")
            nc.sync.dma_start(
                out=fst[:].rearrange("p (ko n) -> p ko n", ko=KO),
                in_=src_ap[:, col0:col0 + ncols].rearrange("(ko p) n -> p ko n", p=128))
            for d in range(NDT):
                ps = mmp.tile([128, ncols], dt.float32, tag="mm")
                for ko in range(KO):                 # weight-stationary inner order
                    for half in range(ncols // 512):
                        nc.tensor.matmul(
                            ps[:, half * 512:(half + 1) * 512],
                            wt[:, ko * DF + d * 128: ko * DF + d * 128 + 128],
                            fst[:, ko * ncols + half * 512: ko * ncols + (half + 1) * 512],
                            start=(ko == 0), stop=(ko == KO - 1))
                nc.vector.tensor_scalar(
                    dst[d][:, dst_col0:dst_col0 + ncols], ps[:],
                    b_col[:, d:d + 1], 0.0, ALU.add, ALU.max)

        project(f1t, 0, L, e1, 0)
        project(f2t, 0, L, e2, 0)          # own batch first (banded phase needs it)

        # ---- banded positive terms (own batch = e2 cols 0:L) -----------
        def boxsum(src_view, r, tag):
            """Return [128, L] view/tile: out[:, j] = sum_{|d|<=r} src[:, j+d] (clipped)."""
            wdt = 2 * r + 1
            pb = band.tile([128, L + 2 * PAD], dt.bfloat16, name=f"pb_{tag}",
                           tag="pb", bufs=2)
            nc.vector.memzero(pb[:])
            nc.vector.tensor_copy(pb[:, PAD:PAD + L], src_view)
            s = {1: pb}
            for p in (2, 4, 8):
                if wdt >= p:
                    sp = band.tile([128, L + 2 * PAD], dt.bfloat16, name=f"s{p}_{tag}",
                                   tag=f"s{p}")
                    h = p // 2
                    n_valid = L + 2 * PAD - p + 1
                    nc.vector.tensor_tensor(
                        sp[:, :n_valid], s[h][:, :n_valid], s[h][:, h:h + n_valid], ALU.add)
                    s[p] = sp
            terms = _box_terms(wdt)
            t0 = PAD - r
            if len(terms) == 1:
                p0, o0 = terms[0]
                return s[p0][:, t0 + o0: t0 + o0 + L]
            acc = band.tile([128, L], dt.bfloat16, name=f"box_{tag}", tag="box", bufs=6)
            p0, o0 = terms[0]
            p1, o1 = terms[1]
            nc.vector.tensor_tensor(acc[:], s[p0][:, t0 + o0: t0 + o0 + L],
                                    s[p1][:, t0 + o1: t0 + o1 + L], ALU.add)
            for p, o in terms[2:]:
                nc.vector.tensor_tensor(acc[:], acc[:], s[p][:, t0 + o: t0 + o + L], ALU.add)
            return acc[:]

        def reduce_group(pairs, out_dram, tag):
            """out_dram[j] = sum over pairs (a,b) and d of (a*b)[d, j]."""
            row = rows.tile([1, L], dt.float32, tag=f"row_{tag}")
            for half in range(L // 512):
                rp = mmp.tile([1, 512], dt.float32, tag="mm", name=f"rp_{tag}_{half}")
                for gi, (a_view, b_view) in enumerate(pairs):
                    prod = prodp.tile([128, 512], dt.bfloat16, tag="prod")
                    nc.vector.tensor_tensor(
                        prod[:], a_view[:, half * 512:(half + 1) * 512],
                        b_view[:, half * 512:(half + 1) * 512], ALU.mult)
                    nc.tensor.matmul(rp[:], ones[:], prod[:],
                                     start=(gi == 0), stop=(gi == len(pairs) - 1))
                nc.vector.tensor_copy(row[:, half * 512:(half + 1) * 512], rp[:])
            nc.sync.dma_start(out=out_dram[:].rearrange("(one n) -> one n", one=1), in_=row[:])

        e2L = [e2[d][:, 0:L] for d in range(NDT)]
        reduce_group([(e1[d][:], e2L[d]) for d in range(NDT)], pos_main, "main")
        if r_self > 0:
            bx1 = [boxsum(e1[d][:], r_self, f"s1_{d}") for d in range(NDT)]
            bx2 = [boxsum(e2L[d], r_self, f"s2_{d}") for d in range(NDT)]
            reduce_group([(e1[d][:], bx1[d]) for d in range(NDT)]
                         + [(e2L[d], bx2[d]) for d in range(NDT)], pos_self, "self")
        else:
            zr = rows.tile([1, L], dt.float32, tag="zr")
            nc.vector.memset(zr[:], 0.0)
            nc.sync.dma_start(out=pos_self[:].rearrange("(one n) -> one n", one=1), in_=zr[:])
        if r_tgt > 0:
            bxt = [boxsum(e2L[d], r_tgt, f"t_{d}") for d in range(NDT)]
            reduce_group([(e1[d][:], bxt[d]) for d in range(NDT)], pos_tgt, "tgt")
        else:
            zr2 = rows.tile([1, L], dt.float32, tag="zr2")
            nc.vector.memset(zr2[:], 0.0)
            nc.sync.dma_start(out=pos_tgt[:].rearrange("(one n) -> one n", one=1), in_=zr2[:])

        # diag bias column layout: [128, 8] with diag[p, a] = pos_main[a*128+p]
        neg_diag = const.tile([128, 8], dt.float32)
        nc.sync.dma_start(out=neg_diag[:], in_=pos_main[:].rearrange("(a p) -> p a", p=128))
        nc.vector.tensor_scalar_mul(neg_diag[:], neg_diag[:], -1.0)

        # ---- rest of e2 projection (chunks 1..7) ------------------------
        for c in range(1, B):
            project(f2t, c * L, L, e2, c * L)

        # ---- sim slab + streaming exp-sum -------------------------------
        ncg = N // CW                           # column groups per row tile
        stot = const.tile([128, 8 * ncg], dt.float32)
        for m in range(8):                      # 128-token row tiles
            for c in range(ncg):                # CW-wide column groups
                ps = mmp.tile([128, CW], dt.float32, tag="mm")
                for d_ in range(NDT):           # weight-stationary inner order
                    for q in range(CW // 512):
                        nc.tensor.matmul(
                            ps[:, q * 512:(q + 1) * 512],
                            e1[d_][:, m * 128:(m + 1) * 128],
                            e2[d_][:, c * CW + q * 512: c * CW + (q + 1) * 512],
                            start=(d_ == 0), stop=(d_ == NDT - 1))
                ex = esc.tile([128, CW], dt.bfloat16, tag="ex")
                nc.scalar.activation(ex[:], ps[:], AF.Exp,
                                     bias=neg_diag[:, m:m + 1], scale=1.0,
                                     accum_out=stot[:, m * ncg + c: m * ncg + c + 1])
        nc.sync.dma_start(out=s_out[:], in_=stot[:])

    nc.compile()
    return nc


def kernel(feature1, feature2, W, b, positive_range_self, positive_range_tgt):
    r_self = int(np.asarray(positive_range_self))
    r_tgt = int(np.asarray(positive_range_tgt))
    assert 0 <= r_self <= PAD and 0 <= r_tgt <= PAD

    key = (r_self, r_tgt)
    if key not in _module_cache:
        _module_cache[key] = _build(r_self, r_tgt)
    nc = _module_cache[key]

    in_maps = _make_in_maps(feature1, feature2, W, b)
    res = bass_utils.run_bass_kernel_spmd(nc, in_maps, list(range(N_CORES)))

    # ---- host combine (fp64) ---------------------------------------------
    j = np.arange(L)
    loss_terms = []
    for i in range(N_CORES):
        r = res.results[i]
        # S groups: stot[p, m*ncg + c]; token j = m*128 + p; sum over c groups
        ncg = N // CW
        S = r["s_out"].astype(np.float64).reshape(128, 8, ncg).sum(axis=2)
        S = S.T.reshape(L)                                   # token j at [j%128, j//128]
        t = np.log(S) - np.log(float(N))                     # negative - diag (diag cancels)
        if r_self > 0:
            cnt = np.minimum(L - 1, j + r_self) - np.maximum(0, j - r_self) + 1.0
            t -= r["pos_self"].astype(np.float64) / cnt
        if r_tgt > 0:
            cnt = np.minimum(L - 1, j + r_tgt) - np.maximum(0, j - r_tgt) + 1.0
            t -= r["pos_tgt"].astype(np.float64) / cnt
        loss_terms.append(t)
    loss = np.mean(np.concatenate(loss_terms))
    return np.float32(loss)


def _make_in_maps(feature1, feature2, W, b):
    bf16 = ml_dtypes.bfloat16
    f1 = np.asarray(feature1, dtype=np.float32)
    f2 = np.asarray(feature2, dtype=np.float32)
    Wr = np.ascontiguousarray(np.asarray(W, dtype=np.float32).astype(bf16))
    bv = np.ascontiguousarray(np.asarray(b, dtype=np.float32))
    f2t_full = f2.reshape(N, DIN).T.astype(bf16)             # [DIN, N]
    in_maps = []
    for i in range(N_CORES):
        f1t_i = np.ascontiguousarray(f1[i].T.astype(bf16))   # [DIN, L]
        f2t_rot = np.ascontiguousarray(np.roll(f2t_full, -i * L, axis=1))
        in_maps.append({"f1t": f1t_i, "f2t": f2t_rot, "w_in": Wr, "b_in": bv})
    return in_maps
